# revision 17
# baseline (speedup 1.0000x reference)
"""Trainium2 Bass kernel for Mistral-style sliding-window GQA attention.

Problem: B=2, T=2048, C=2048, 32 q heads / 8 kv heads, head_dim=64,
sliding causal window 1024, RoPE, fp32.

Sharding (sequence-parallel, no cross-core communication):
  core c in 0..7 handles batch b=c//4 and contiguous 512-row chunk k=c%4.
  Each core computes q for its 512 rows, k/v for its rows plus a 1024-row
  halo (zero-padded before t=0), full attention for its rows over all 32
  heads, and the output projection for its rows.  Host gathers by
  concatenation only.

Device program details:
  - float32r (single-pass fp32 PE mode, ~1.5e-4 matmul error) for all
    matmul operands; PSUM accumulation stays fp32.
  - x is transposed on host; RoPE cos/sin tables and masks are host inputs.
  - scores are built in S^T = [key, query] layout, with the 4 query heads of
    each kv group packed side-by-side in the moving operand (N=512 matmuls).
  - PV uses V as the stationary operand: out^T = V_ext^T @ P^T accumulates
    [65, 4x128] per (group, q-tile); row 64 (from the validity column of
    V_ext) is the softmax denominator, and rows 0..63 are already in the
    aT=[d, t] layout the output projection needs - no transposes anywhere.
  - no max-subtraction in softmax: inputs are N(0,1)-scaled so |scores/8|
    stays ~15; exp is safe in fp32.
  - Q projection runs as four 8-head quarter-sweeps interleaved with
    attention over the heads already produced, so ACT-bound softmax overlaps
    PE-bound projection.
  - invalid (zero-padded halo) keys get exp(0)=1 scores but contribute zero
    to both PV numerator and the validity-column denominator.
"""

import os
import numpy as np

import concourse.bass as bass
import concourse.mybir as mybir
import concourse.tile as tile
from concourse import bacc
from concourse.bass_utils import run_bass_kernel_spmd

B, T, C = 2, 2048, 2048
NH, NKV, D = 32, 8, 64
REP = NH // NKV
WIN = 1024
CH = 512          # q rows per core
KVR = CH + WIN    # kv rows per core (with halo)
NCORE = 8
DT = mybir.dt.float32
F32R = mybir.dt.float32r
SCALE = 1.0 / np.sqrt(np.float32(D))
ROPE_BASE = 10000.0

FD = T // 128     # 16 contraction tiles of the model dim
NQT = CH // 128   # 4 q tiles per chunk
NKB = KVR // 128  # 12 kv blocks per core
NWB = 9           # kv blocks in the window of one q tile
VW = 65           # v_ext width per kv block (64 dims + validity column)
VP = NKB * VW     # per-head v_ext pitch (780)


def _rope_write(nc, pool, out_ap, ps, cosw, ssinw, n, swap_engine=None):
    """out = ps*cos + rot_half(ps)*sin on a [128, n] 2-head-packed tile.

    ssinw rows carry the rotate-half signs (rows 0-31/64-95 negated) and any
    folded scale; cosw carries the same scale.  out_ap is either one [128, n]
    AP or a list of two ([64, n] AP) halves receiving rows 0:64 / 64:128.

    If swap_engine is given (an idle PSUM-capable engine, e.g. nc.scalar),
    the rotate-half shuffle is materialized there with 4 quarter copies and
    the vector engine does only 3 full-width ops; otherwise the vector
    engine does 4 quarter multiplies + 2 full ops.
    """
    if swap_engine is not None:
        sw = pool.tile([128, n], DT, tag="rope_sw", name="rope_sw")
        swap_engine.copy(sw[0:32, :], ps[32:64, :])
        swap_engine.copy(sw[32:64, :], ps[0:32, :])
        swap_engine.copy(sw[64:96, :], ps[96:128, :])
        swap_engine.copy(sw[96:128, :], ps[64:96, :])
        t2 = pool.tile([128, n], DT, tag="rope_t2", name="rope_t2")
        nc.vector.tensor_mul(t2[:], sw[:], ssinw[:])
    else:
        t2 = pool.tile([128, n], DT, tag="rope_t2", name="rope_t2")
        nc.vector.tensor_mul(t2[0:32, :], ps[32:64, :], ssinw[0:32, :])
        nc.vector.tensor_mul(t2[32:64, :], ps[0:32, :], ssinw[32:64, :])
        nc.vector.tensor_mul(t2[64:96, :], ps[96:128, :], ssinw[64:96, :])
        nc.vector.tensor_mul(t2[96:128, :], ps[64:96, :], ssinw[96:128, :])
    t1 = pool.tile([128, n], DT, tag="rope_t1", name="rope_t1")
    nc.vector.tensor_mul(t1[:], ps[:], cosw[:])
    if isinstance(out_ap, list):
        for i, half in enumerate(out_ap):
            nc.vector.tensor_add(half, t1[64 * i:64 * (i + 1), :],
                                 t2[64 * i:64 * (i + 1), :])
    else:
        nc.vector.tensor_add(out_ap, t1[:], t2[:])


def build_program():
    nc = bacc.Bacc("TRN2", target_bir_lowering=False, debug=False,
                   num_devices=NCORE)

    xkv_d = nc.dram_tensor("xkv", [C, KVR], F32R, kind="ExternalInput")
    wq_d = nc.dram_tensor("wq", [C, NH * D], F32R, kind="ExternalInput")
    wk_d = nc.dram_tensor("wk", [C, NKV * D], F32R, kind="ExternalInput")
    wv_d = nc.dram_tensor("wv", [C, NKV * D], F32R, kind="ExternalInput")
    wo_d = nc.dram_tensor("wo", [NH * D, C], F32R, kind="ExternalInput")
    rqc_d = nc.dram_tensor("rope_q_cos", [128, CH], DT, kind="ExternalInput")
    rqs_d = nc.dram_tensor("rope_q_sin", [128, CH], DT, kind="ExternalInput")
    rkc_d = nc.dram_tensor("rope_k_cos", [128, KVR], DT, kind="ExternalInput")
    rks_d = nc.dram_tensor("rope_k_sin", [128, KVR], DT, kind="ExternalInput")
    kvv_d = nc.dram_tensor("kvvalid", [128, NKB], F32R, kind="ExternalInput")
    mw_d = nc.dram_tensor("mask_win8", [128, 1024], F32R, kind="ExternalInput")
    mc_d = nc.dram_tensor("mask_causal8", [128, 1024], F32R,
                          kind="ExternalInput")
    out_d = nc.dram_tensor("out", [CH, C], DT, kind="ExternalOutput")

    with tile.TileContext(nc) as tc:
        with (
            tc.tile_pool(name="const", bufs=1) as cpool,
            tc.tile_pool(name="qT", bufs=1) as qT_pool,
            tc.tile_pool(name="kT", bufs=1) as kT_pool,
            tc.tile_pool(name="vext", bufs=1) as v_pool,
        ):
            # ---- constants (small, persistent) ----
            mask_win = cpool.tile([128, 1024], F32R, tag="mw", name="mask_win")
            nc.scalar.dma_start(mask_win[:], mw_d[:, :])
            mask_causal = cpool.tile([128, 1024], F32R, tag="mc",
                                     name="mask_causal")
            nc.scalar.dma_start(mask_causal[:], mc_d[:, :])
            kvv = cpool.tile([128, NKB], F32R, tag="kvv", name="kvv")
            nc.scalar.dma_start(kvv[:], kvv_d[:, :])

            # qT: [d, t] grouped by kv head.  Tile j rows 0:64 = group 2j
            # (its 4 heads side by side, 512 cols each), rows 64:128 =
            # group 2j+1, so QK lhsT and rhs share a base partition.
            qT = [qT_pool.tile([128, REP * CH], F32R, tag=f"qT{i}",
                               name=f"qT{i}") for i in range(NKV // 2)]
            # kT: [d, t] packed 2 kv heads per tile.
            kT = [kT_pool.tile([128, KVR], F32R, tag=f"kT{i}", name=f"kT{i}")
                  for i in range(NKV // 2)]
            # vext: one tile, head kvh at pitch VP; per block 64 dims+validity
            vext = v_pool.tile([128, NKV * VP], F32R, tag="vext", name="vext")

            # ================= KV projection =================
            with (
                tc.tile_pool(name="rk_tab", bufs=1) as rk_pool,
                tc.tile_pool(name="wk_res", bufs=1) as wk_pool,
                tc.tile_pool(name="wv_res", bufs=1) as wv_pool,
                tc.tile_pool(name="xkv_s", bufs=4) as xkv_pool,
                tc.tile_pool(name="rope_tmp", bufs=3) as rtmp,
                tc.tile_pool(name="ps_kv", bufs=1, space="PSUM") as ps_kv,
            ):
                rkc = rk_pool.tile([128, KVR], DT, tag="rkc", name="rkc")
                nc.scalar.dma_start(rkc[:], rkc_d[:, :])
                rks = rk_pool.tile([128, KVR], DT, tag="rks", name="rks")
                nc.scalar.dma_start(rks[:], rks_d[:, :])
                wkt = {}
                wvt = {}
                for ci in range(FD):
                    wkt[ci] = wk_pool.tile([128, NKV * D], F32R,
                                           tag=f"wk{ci}", name=f"wk{ci}")
                    nc.scalar.dma_start(
                        wkt[ci][:], wk_d[128 * ci:128 * (ci + 1), :])
                    wvt[ci] = wv_pool.tile([128, NKV * D], F32R,
                                           tag=f"wv{ci}", name=f"wv{ci}")
                    nc.scalar.dma_start(
                        wvt[ci][:], wv_d[128 * ci:128 * (ci + 1), :])

                NTH = 3          # x-column thirds
                QW = KVR // NTH  # 512 columns per third
                for qu in range(NTH):
                    qs = QW * qu
                    kps = [ps_kv.tile([128, QW], DT, tag=f"kps{m}",
                                      name=f"kps{m}")
                           for m in range(NKV // 2)]
                    vps = [ps_kv.tile([128, NKV * D], DT, tag=f"vps{st}",
                                      name=f"vps{st}")
                           for st in range(QW // 128)]
                    for ci in range(FD):
                        xt = xkv_pool.tile([128, QW], F32R, tag="xkv",
                                           name="xkv")
                        nc.sync.dma_start(
                            xt[:], xkv_d[128 * ci:128 * (ci + 1), qs:qs + QW])
                        for m in range(NKV // 2):
                            nc.tensor.matmul(
                                kps[m][:], wkt[ci][:, 128 * m:128 * (m + 1)],
                                xt[:], start=(ci == 0), stop=(ci == FD - 1))
                        for st in range(QW // 128):
                            nc.tensor.matmul(
                                vps[st][:], xt[:, 128 * st:128 * (st + 1)],
                                wvt[ci][:], start=(ci == 0),
                                stop=(ci == FD - 1))
                    for m in range(NKV // 2):
                        _rope_write(nc, rtmp, kT[m][:, qs:qs + QW],
                                    kps[m][:], rkc[:, qs:qs + QW],
                                    rks[:, qs:qs + QW], QW,
                                    swap_engine=nc.scalar)
                    for st in range(QW // 128):
                        tl = (QW // 128) * qu + st   # kv block 0..11
                        # v data for all 8 heads in one strided copy
                        nc.scalar.copy(
                            vext[:].rearrange("p (h b w) -> p h b w",
                                              h=NKV, b=NKB)[:, :, tl, 0:D],
                            vps[st][:].rearrange("p (h d) -> p h d", h=NKV))
                    # validity columns for this third's blocks, all heads
                    t0 = (QW // 128) * qu
                    nc.scalar.copy(
                        vext[:].rearrange("p (h b w) -> p h b w",
                                          h=NKV, b=NKB)[
                                              :, :, t0:t0 + QW // 128,
                                              D:D + 1],
                        kvv[:, t0:t0 + QW // 128].rearrange(
                            "p (o b) -> p o b", o=1).to_broadcast(
                                (128, NKV, QW // 128)))

            # ====== interleaved Q projection + attention ladder ======
            with (
                tc.tile_pool(name="aT", bufs=1) as aT_pool,
                tc.tile_pool(name="rq_tab", bufs=1) as rq_pool,
                tc.tile_pool(name="wq_s", bufs=4) as wq_pool,
                tc.tile_pool(name="xq_s", bufs=6) as xq_pool,
                tc.tile_pool(name="rope_tmp_q", bufs=3) as rtmpq,
                tc.tile_pool(name="pt", bufs=6) as pt_pool,
                tc.tile_pool(name="att_small", bufs=4) as sm_pool,
                tc.tile_pool(name="ps_att", bufs=1, space="PSUM") as ps_att,
            ):
                aT = [aT_pool.tile([128, CH], F32R, tag=f"aT{i}",
                                   name=f"aT{i}") for i in range(NH // 2)]
                rqc = rq_pool.tile([128, CH], DT, tag="rqc", name="rqc")
                nc.scalar.dma_start(rqc[:], rqc_d[:, :])
                rqs = rq_pool.tile([128, CH], DT, tag="rqs", name="rqs")
                nc.scalar.dma_start(rqs[:], rqs_d[:, :])

                def q_quarter(sweep):
                    # projects heads 8*sweep .. 8*sweep+7 (psum m-tiles
                    # 4*sweep .. 4*sweep+3)
                    qps = [ps_q.tile([128, CH], DT, tag=f"qps{m4}",
                                     name=f"qps{m4}") for m4 in range(4)]
                    for ci in range(FD):
                        xt = xq_pool.tile([128, CH], F32R, tag="xq", name="xq")
                        nc.sync.dma_start(
                            xt[:], xkv_d[128 * ci:128 * (ci + 1),
                                         WIN:WIN + CH])
                        wt = wq_pool.tile([128, 512], F32R, tag="wq",
                                          name="wq")
                        nc.sync.dma_start(
                            wt[:], wq_d[128 * ci:128 * (ci + 1),
                                        512 * sweep:512 * (sweep + 1)])
                        for m4 in range(4):
                            nc.tensor.matmul(qps[m4][:],
                                             wt[:, 128 * m4:128 * (m4 + 1)],
                                             xt[:], start=(ci == 0),
                                             stop=(ci == FD - 1))
                    for m4 in range(4):
                        m = 4 * sweep + m4
                        boff = 64 * ((m // 2) % 2)
                        tau = m // 4
                        c0 = 512 * (2 * (m % 2))
                        _rope_write(nc, rtmpq,
                                    [qT[tau][boff:boff + 64, c0:c0 + 512],
                                     qT[tau][boff:boff + 64,
                                             c0 + 512:c0 + 1024]],
                                    qps[m4][:], rqc[:], rqs[:], CH)

                def attention_pair(gp):
                    for g in (2 * gp, 2 * gp + 1):
                        kTt, koff = kT[g // 2], 64 * (g % 2)
                        qTg = qT[g // 2]
                        qr = qTg[koff:koff + 64, :].rearrange(
                            "p (r t) -> p r t", r=REP)
                        for qtp in range(NQT // 2):
                            qa, qb = 2 * qtp, 2 * qtp + 1
                            OTa = ps_att.tile([65, REP * 128], DT,
                                              tag="OT", name="OTa", bufs=2)
                            OTb = ps_att.tile([65, REP * 128], DT,
                                              tag="OT", name="OTb", bufs=2)
                            for lk in range(NWB):
                                ST = ps_att.tile([128, 2 * REP * 128], DT,
                                                 tag="ST", name="ST", bufs=1)
                                STv = ST.rearrange("p (u r t) -> p u r t",
                                                   u=2, r=REP)
                                nc.tensor.matmul(
                                    STv[:, 0], kTt[koff:koff + 64,
                                                   128 * (qa + lk):
                                                   128 * (qa + lk + 1)],
                                    qr[:, :, 128 * qa:128 * (qa + 1)],
                                    start=True, stop=True)
                                nc.tensor.matmul(
                                    STv[:, 1], kTt[koff:koff + 64,
                                                   128 * (qb + lk):
                                                   128 * (qb + lk + 1)],
                                    qr[:, :, 128 * qb:128 * (qb + 1)],
                                    start=True, stop=True)
                                PT = pt_pool.tile([128, 2 * REP * 128], F32R,
                                                  tag="PT", name="PT", bufs=3)
                                nc.scalar.activation(
                                    PT[:], ST[:],
                                    mybir.ActivationFunctionType.Exp)
                                if lk == 0:
                                    nc.gpsimd.tensor_mul(PT[:], PT[:],
                                                         mask_win[:])
                                elif lk == NWB - 1:
                                    nc.gpsimd.tensor_mul(PT[:], PT[:],
                                                         mask_causal[:])
                                nc.tensor.matmul(
                                    OTa[:],
                                    vext[:, VP * g + VW * (qa + lk):
                                         VP * g + VW * (qa + lk + 1)],
                                    PT[:, 0:512], start=(lk == 0),
                                    stop=(lk == NWB - 1))
                                nc.tensor.matmul(
                                    OTb[:],
                                    vext[:, VP * g + VW * (qb + lk):
                                         VP * g + VW * (qb + lk + 1)],
                                    PT[:, 512:1024], start=(lk == 0),
                                    stop=(lk == NWB - 1))
                            for qt, OT in ((qa, OTa), (qb, OTb)):
                                rcp = sm_pool.tile([1, REP * 128], DT,
                                                   tag="rcp", name="rcp")
                                nc.vector.reciprocal(rcp[:], OT[64:65, :])
                                rcpb = sm_pool.tile([64, REP * 128], DT,
                                                    tag="rcpb", name="rcpb")
                                nc.gpsimd.partition_broadcast(rcpb[:], rcp[:])
                                for r in range(REP):
                                    h = REP * g + r
                                    nc.vector.tensor_mul(
                                        aT[h // 2][64 * (h % 2):
                                                   64 * (h % 2) + 64,
                                                   128 * qt:128 * (qt + 1)],
                                        OT[0:64, 128 * r:128 * (r + 1)],
                                        rcpb[:, 128 * r:128 * (r + 1)])

                with tc.tile_pool(name="ps_q", bufs=1, space="PSUM") as ps_q:
                    for sweep in range(4):
                        q_quarter(sweep)
                        if sweep >= 1:
                            attention_pair(sweep - 1)
                attention_pair(3)

                # ================= output projection =================
                with (
                    tc.tile_pool(name="wo_s", bufs=8) as wo_pool,
                    tc.tile_pool(name="ostage", bufs=3) as ostage,
                    tc.tile_pool(name="ps_o", bufs=1, space="PSUM") as ps_o,
                ):
                    for oc in range(4):
                        ops = [ps_o.tile([128, 512], DT, tag=f"ops{tt}",
                                         name=f"ops{tt}")
                               for tt in range(NQT)]
                        for k in range(FD):
                            wot = wo_pool.tile([128, 512], F32R, tag="wo",
                                               name="wo")
                            eng = nc.scalar if k % 2 else nc.sync
                            eng.dma_start(
                                wot[:], wo_d[128 * k:128 * (k + 1),
                                             512 * oc:512 * (oc + 1)])
                            for tt in range(NQT):
                                nc.tensor.matmul(
                                    ops[tt][:],
                                    aT[k][:, 128 * tt:128 * (tt + 1)],
                                    wot[:], start=(k == 0),
                                    stop=(k == FD - 1))
                        for tt in range(NQT):
                            st = ostage.tile([128, 512], DT, tag="stage",
                                             name="stage")
                            nc.vector.tensor_copy(st[:], ops[tt][:])
                            nc.gpsimd.dma_start(
                                out_d[128 * tt:128 * (tt + 1),
                                      512 * oc:512 * (oc + 1)], st[:])

    nc.compile()
    return nc


def _rope_tables(t_idx, scale):
    """cos/sin tables in [d, t] layout, 2-head packed to 128 partitions.

    Rows 0-63 and 64-127 identical; sin rows 0-31 (and 64-95) carry the
    rotate-half minus sign."""
    inv_freq = 1.0 / (ROPE_BASE ** (np.arange(0, D, 2, dtype=np.float64) / D))
    ang = t_idx[None, :] * inv_freq[:, None]          # [32, n]
    cos1 = np.cos(ang)
    sin1 = np.sin(ang)
    cos64 = np.concatenate([cos1, cos1], 0) * scale   # [64, n]
    sin64 = np.concatenate([-sin1, sin1], 0) * scale  # [64, n] signed
    return (np.tile(cos64, (2, 1)).astype(np.float32),
            np.tile(sin64, (2, 1)).astype(np.float32))


def make_in_maps(x, Wq, Wk, Wv, Wo):
    x = np.asarray(x, np.float32)
    ins = []
    i = np.arange(128)
    masks = {
        "mask_win8": np.tile((i[:, None] > i[None, :]).astype(np.float32),
                             (1, 2 * REP)),
        "mask_causal8": np.tile((i[:, None] <= i[None, :]).astype(np.float32),
                                (1, 2 * REP)),
    }
    for c in range(NCORE):
        b, ch = divmod(c, 4)
        r0 = CH * ch
        kv0 = r0 - WIN
        xT = np.ascontiguousarray(x[b].T)             # [C, T]
        xkv = np.zeros((C, KVR), np.float32)
        pad = max(0, -kv0)
        xkv[:, pad:] = xT[:, kv0 + pad:r0 + CH]
        qc, qs = _rope_tables(np.arange(r0, r0 + CH, dtype=np.float64), SCALE)
        kc, ks = _rope_tables(np.arange(kv0, r0 + CH, dtype=np.float64), 1.0)
        kvvalid = np.zeros((128, NKB), np.float32)
        for lk in range(NKB):
            kvvalid[:, lk] = (kv0 + 128 * lk + i >= 0).astype(np.float32)
        ins.append({
            "xkv": xkv,
            "wq": np.ascontiguousarray(Wq, np.float32),
            "wk": np.ascontiguousarray(Wk, np.float32),
            "wv": np.ascontiguousarray(Wv, np.float32),
            "wo": np.ascontiguousarray(Wo, np.float32),
            "rope_q_cos": qc, "rope_q_sin": qs,
            "rope_k_cos": kc, "rope_k_sin": ks,
            "kvvalid": kvvalid,
            **masks,
        })
    return ins


_PROG_CACHE = {}


def get_program():
    if "nc" not in _PROG_CACHE:
        _PROG_CACHE["nc"] = build_program()
    return _PROG_CACHE["nc"]


def kernel(x, Wq, Wk, Wv, Wo):
    nc = get_program()
    ins = make_in_maps(x, Wq, Wk, Wv, Wo)
    res = run_bass_kernel_spmd(nc, ins, list(range(NCORE)))
    out = np.empty((B, T, C), np.float32)
    for c in range(NCORE):
        b, ch = divmod(c, 4)
        out[b, CH * ch:CH * (ch + 1), :] = res.results[c]["out"]
    return out


# revision 18
# speedup vs baseline: 1.2203x; 1.2203x over previous
"""Trainium2 Bass kernel for Mistral-style sliding-window GQA attention.

Problem: B=2, T=2048, C=2048, 32 q heads / 8 kv heads, head_dim=64,
sliding causal window 1024, RoPE, fp32.

Sharding (sequence-parallel, no cross-core communication):
  core c in 0..7 handles batch b=c//4 and contiguous 512-row chunk k=c%4.
  Each core computes q for its 512 rows, k/v for its rows plus a 1024-row
  halo (zero-padded before t=0), full attention for its rows over all 32
  heads, and the output projection for its rows.  Host gathers by
  concatenation only.

Device program details:
  - float32r (single-pass fp32 PE mode, ~1.5e-4 matmul error) for all
    matmul operands; PSUM accumulation stays fp32.
  - x is transposed on host; RoPE cos/sin tables and masks are host inputs.
  - scores are built in S^T = [key, query] layout, with the 4 query heads of
    each kv group packed side-by-side in the moving operand (N=512 matmuls).
  - PV uses V as the stationary operand: out^T = V_ext^T @ P^T accumulates
    [65, 4x128] per (group, q-tile); row 64 (from the validity column of
    V_ext) is the softmax denominator, and rows 0..63 are already in the
    aT=[d, t] layout the output projection needs - no transposes anywhere.
  - no max-subtraction in softmax: inputs are N(0,1)-scaled so |scores/8|
    stays ~15; exp is safe in fp32.
  - Q projection runs as four 8-head quarter-sweeps interleaved with
    attention over the heads already produced, so ACT-bound softmax overlaps
    PE-bound projection.
  - invalid (zero-padded halo) keys get exp(0)=1 scores but contribute zero
    to both PV numerator and the validity-column denominator.
"""

import os
import numpy as np

import concourse.bass as bass
import concourse.mybir as mybir
import concourse.tile as tile
from concourse import bacc
from concourse.bass_utils import run_bass_kernel_spmd

B, T, C = 2, 2048, 2048
NH, NKV, D = 32, 8, 64
REP = NH // NKV
WIN = 1024
CH = 512          # q rows per core
KVR = CH + WIN    # kv rows per core (with halo)
NCORE = 8
DT = mybir.dt.float32
F32R = mybir.dt.float32r
SCALE = 1.0 / np.sqrt(np.float32(D))
ROPE_BASE = 10000.0

FD = T // 128     # 16 contraction tiles of the model dim
NQT = CH // 128   # 4 q tiles per chunk
NKB = KVR // 128  # 12 kv blocks per core
NWB = 9           # kv blocks in the window of one q tile
VW = 65           # v_ext width per kv block (64 dims + validity column)
VP = NKB * VW     # per-head v_ext pitch (780)


def _rope_write(nc, pool, out_ap, ps, cosw, ssinw, n, swap_engine=None):
    """out = ps*cos + rot_half(ps)*sin on a [128, n] 2-head-packed tile.

    ssinw rows carry the rotate-half signs (rows 0-31/64-95 negated) and any
    folded scale; cosw carries the same scale.  out_ap is either one [128, n]
    AP or a list of two ([64, n] AP) halves receiving rows 0:64 / 64:128.

    If swap_engine is given (an idle PSUM-capable engine, e.g. nc.scalar),
    the rotate-half shuffle is materialized there with 4 quarter copies and
    the vector engine does only 3 full-width ops; otherwise the vector
    engine does 4 quarter multiplies + 2 full ops.
    """
    if swap_engine is not None:
        sw = pool.tile([128, n], DT, tag="rope_sw", name="rope_sw")
        swap_engine.copy(sw[0:32, :], ps[32:64, :])
        swap_engine.copy(sw[32:64, :], ps[0:32, :])
        swap_engine.copy(sw[64:96, :], ps[96:128, :])
        swap_engine.copy(sw[96:128, :], ps[64:96, :])
        t2 = pool.tile([128, n], DT, tag="rope_t2", name="rope_t2")
        nc.vector.tensor_mul(t2[:], sw[:], ssinw[:])
    else:
        t2 = pool.tile([128, n], DT, tag="rope_t2", name="rope_t2")
        nc.vector.tensor_mul(t2[0:32, :], ps[32:64, :], ssinw[0:32, :])
        nc.vector.tensor_mul(t2[32:64, :], ps[0:32, :], ssinw[32:64, :])
        nc.vector.tensor_mul(t2[64:96, :], ps[96:128, :], ssinw[64:96, :])
        nc.vector.tensor_mul(t2[96:128, :], ps[64:96, :], ssinw[96:128, :])
    t1 = pool.tile([128, n], DT, tag="rope_t1", name="rope_t1")
    nc.vector.tensor_mul(t1[:], ps[:], cosw[:])
    if isinstance(out_ap, list):
        for i, half in enumerate(out_ap):
            nc.vector.tensor_add(half, t1[64 * i:64 * (i + 1), :],
                                 t2[64 * i:64 * (i + 1), :])
    else:
        nc.vector.tensor_add(out_ap, t1[:], t2[:])


def build_program():
    nc = bacc.Bacc("TRN2", target_bir_lowering=False, debug=False,
                   num_devices=NCORE)

    xkv_d = nc.dram_tensor("xkv", [C, KVR], F32R, kind="ExternalInput")
    wq_d = nc.dram_tensor("wq", [C, NH * D], F32R, kind="ExternalInput")
    wk_d = nc.dram_tensor("wk", [C, NKV * D], F32R, kind="ExternalInput")
    wv_d = nc.dram_tensor("wv", [C, NKV * D], F32R, kind="ExternalInput")
    wo_d = nc.dram_tensor("wo", [NH * D, C], F32R, kind="ExternalInput")
    rqc_d = nc.dram_tensor("rope_q_cos", [128, CH], DT, kind="ExternalInput")
    rqs_d = nc.dram_tensor("rope_q_sin", [128, CH], DT, kind="ExternalInput")
    rkc_d = nc.dram_tensor("rope_k_cos", [128, KVR], DT, kind="ExternalInput")
    rks_d = nc.dram_tensor("rope_k_sin", [128, KVR], DT, kind="ExternalInput")
    kvv_d = nc.dram_tensor("kvvalid", [128, NKB], F32R, kind="ExternalInput")
    mw_d = nc.dram_tensor("mask_win8", [128, 512], F32R, kind="ExternalInput")
    mc_d = nc.dram_tensor("mask_causal8", [128, 512], F32R,
                          kind="ExternalInput")
    out_d = nc.dram_tensor("out", [CH, C], DT, kind="ExternalOutput")

    with tile.TileContext(nc) as tc:
        with (
            tc.tile_pool(name="const", bufs=1) as cpool,
            tc.tile_pool(name="qT", bufs=1) as qT_pool,
            tc.tile_pool(name="kT", bufs=1) as kT_pool,
            tc.tile_pool(name="vext", bufs=1) as v_pool,
        ):
            # ---- constants (small, persistent) ----
            mask_win = cpool.tile([128, 512], F32R, tag="mw", name="mask_win")
            nc.scalar.dma_start(mask_win[:], mw_d[:, :])
            mask_causal = cpool.tile([128, 512], F32R, tag="mc",
                                     name="mask_causal")
            nc.scalar.dma_start(mask_causal[:], mc_d[:, :])
            kvv = cpool.tile([128, NKB], F32R, tag="kvv", name="kvv")
            nc.scalar.dma_start(kvv[:], kvv_d[:, :])

            # qT: [d, t] grouped by kv head.  Tile j rows 0:64 = group 2j
            # (its 4 heads side by side, 512 cols each), rows 64:128 =
            # group 2j+1, so QK lhsT and rhs share a base partition.
            qT = [qT_pool.tile([128, REP * CH], F32R, tag=f"qT{i}",
                               name=f"qT{i}") for i in range(NKV // 2)]
            # kT: [d, t] packed 2 kv heads per tile.
            kT = [kT_pool.tile([128, KVR], F32R, tag=f"kT{i}", name=f"kT{i}")
                  for i in range(NKV // 2)]
            # vext: one tile, head kvh at pitch VP; per block 64 dims+validity
            vext = v_pool.tile([128, NKV * VP], F32R, tag="vext", name="vext")

            # ================= KV projection =================
            with (
                tc.tile_pool(name="rk_tab", bufs=1) as rk_pool,
                tc.tile_pool(name="wk_res", bufs=1) as wk_pool,
                tc.tile_pool(name="wv_res", bufs=1) as wv_pool,
                tc.tile_pool(name="xkv_s", bufs=4) as xkv_pool,
                tc.tile_pool(name="rope_tmp", bufs=3) as rtmp,
                tc.tile_pool(name="ps_kv", bufs=1, space="PSUM") as ps_kv,
            ):
                rkc = rk_pool.tile([128, KVR], DT, tag="rkc", name="rkc")
                nc.scalar.dma_start(rkc[:], rkc_d[:, :])
                rks = rk_pool.tile([128, KVR], DT, tag="rks", name="rks")
                nc.scalar.dma_start(rks[:], rks_d[:, :])
                wkt = {}
                wvt = {}
                for ci in range(FD):
                    wkt[ci] = wk_pool.tile([128, NKV * D], F32R,
                                           tag=f"wk{ci}", name=f"wk{ci}")
                    nc.scalar.dma_start(
                        wkt[ci][:], wk_d[128 * ci:128 * (ci + 1), :])
                    wvt[ci] = wv_pool.tile([128, NKV * D], F32R,
                                           tag=f"wv{ci}", name=f"wv{ci}")
                    nc.scalar.dma_start(
                        wvt[ci][:], wv_d[128 * ci:128 * (ci + 1), :])

                NTH = 3          # x-column thirds
                QW = KVR // NTH  # 512 columns per third
                for qu in range(NTH):
                    qs = QW * qu
                    kps = [ps_kv.tile([128, QW], DT, tag=f"kps{m}",
                                      name=f"kps{m}")
                           for m in range(NKV // 2)]
                    vps = [ps_kv.tile([128, NKV * D], DT, tag=f"vps{st}",
                                      name=f"vps{st}")
                           for st in range(QW // 128)]
                    for ci in range(FD):
                        xt = xkv_pool.tile([128, QW], F32R, tag="xkv",
                                           name="xkv")
                        nc.sync.dma_start(
                            xt[:], xkv_d[128 * ci:128 * (ci + 1), qs:qs + QW])
                        for m in range(NKV // 2):
                            nc.tensor.matmul(
                                kps[m][:], wkt[ci][:, 128 * m:128 * (m + 1)],
                                xt[:], start=(ci == 0), stop=(ci == FD - 1))
                        for st in range(QW // 128):
                            nc.tensor.matmul(
                                vps[st][:], xt[:, 128 * st:128 * (st + 1)],
                                wvt[ci][:], start=(ci == 0),
                                stop=(ci == FD - 1))
                    for m in range(NKV // 2):
                        _rope_write(nc, rtmp, kT[m][:, qs:qs + QW],
                                    kps[m][:], rkc[:, qs:qs + QW],
                                    rks[:, qs:qs + QW], QW,
                                    swap_engine=nc.scalar)
                    for st in range(QW // 128):
                        tl = (QW // 128) * qu + st   # kv block 0..11
                        # v data for all 8 heads in one strided copy
                        nc.scalar.copy(
                            vext[:].rearrange("p (h b w) -> p h b w",
                                              h=NKV, b=NKB)[:, :, tl, 0:D],
                            vps[st][:].rearrange("p (h d) -> p h d", h=NKV))
                    # validity columns for this third's blocks, all heads
                    t0 = (QW // 128) * qu
                    nc.scalar.copy(
                        vext[:].rearrange("p (h b w) -> p h b w",
                                          h=NKV, b=NKB)[
                                              :, :, t0:t0 + QW // 128,
                                              D:D + 1],
                        kvv[:, t0:t0 + QW // 128].rearrange(
                            "p (o b) -> p o b", o=1).to_broadcast(
                                (128, NKV, QW // 128)))

            # ====== interleaved Q projection + attention ladder ======
            with (
                tc.tile_pool(name="aT", bufs=1) as aT_pool,
                tc.tile_pool(name="rq_tab", bufs=1) as rq_pool,
                tc.tile_pool(name="wq_s", bufs=4) as wq_pool,
                tc.tile_pool(name="xq_s", bufs=6) as xq_pool,
                tc.tile_pool(name="rope_tmp_q", bufs=3) as rtmpq,
                tc.tile_pool(name="pt", bufs=6) as pt_pool,
                tc.tile_pool(name="att_small", bufs=4) as sm_pool,
                tc.tile_pool(name="ps_att", bufs=1, space="PSUM") as ps_att,
            ):
                aT = [aT_pool.tile([128, CH], F32R, tag=f"aT{i}",
                                   name=f"aT{i}") for i in range(NH // 2)]
                rqc = rq_pool.tile([128, CH], DT, tag="rqc", name="rqc")
                nc.scalar.dma_start(rqc[:], rqc_d[:, :])
                rqs = rq_pool.tile([128, CH], DT, tag="rqs", name="rqs")
                nc.scalar.dma_start(rqs[:], rqs_d[:, :])

                def q_quarter(sweep):
                    # projects heads 8*sweep .. 8*sweep+7 (psum m-tiles
                    # 4*sweep .. 4*sweep+3)
                    qps = [ps_q.tile([128, CH], DT, tag=f"qps{m4}",
                                     name=f"qps{m4}") for m4 in range(4)]
                    for ci in range(FD):
                        xt = xq_pool.tile([128, CH], F32R, tag="xq", name="xq")
                        nc.sync.dma_start(
                            xt[:], xkv_d[128 * ci:128 * (ci + 1),
                                         WIN:WIN + CH])
                        wt = wq_pool.tile([128, 512], F32R, tag="wq",
                                          name="wq")
                        nc.sync.dma_start(
                            wt[:], wq_d[128 * ci:128 * (ci + 1),
                                        512 * sweep:512 * (sweep + 1)])
                        for m4 in range(4):
                            nc.tensor.matmul(qps[m4][:],
                                             wt[:, 128 * m4:128 * (m4 + 1)],
                                             xt[:], start=(ci == 0),
                                             stop=(ci == FD - 1))
                    for m4 in range(4):
                        m = 4 * sweep + m4
                        boff = 64 * ((m // 2) % 2)
                        tau = m // 4
                        c0 = 512 * (2 * (m % 2))
                        _rope_write(nc, rtmpq,
                                    [qT[tau][boff:boff + 64, c0:c0 + 512],
                                     qT[tau][boff:boff + 64,
                                             c0 + 512:c0 + 1024]],
                                    qps[m4][:], rqc[:], rqs[:], CH)

                def attention_pair(gp):
                    for g in (2 * gp, 2 * gp + 1):
                        kTt, koff = kT[g // 2], 64 * (g % 2)
                        qTg = qT[g // 2]
                        for qt in range(NQT):
                            qv = qTg[koff:koff + 64, :].rearrange(
                                "p (r t) -> p r t", r=REP)[
                                    :, :, 128 * qt:128 * (qt + 1)]
                            OT = ps_att.tile([65, REP * 128], DT,
                                             tag="OT", name="OT", bufs=2)
                            for lk in range(NWB):
                                kb = qt + lk
                                ST = ps_att.tile([128, REP * 128], DT,
                                                 tag="ST", name="ST", bufs=2)
                                nc.tensor.matmul(
                                    ST.rearrange("p (r t) -> p r t", r=REP),
                                    kTt[koff:koff + 64,
                                        128 * kb:128 * (kb + 1)],
                                    qv, start=True, stop=True)
                                PT = pt_pool.tile([128, REP * 128], F32R,
                                                  tag="PT", name="PT", bufs=6)
                                nc.scalar.activation(
                                    PT[:], ST[:],
                                    mybir.ActivationFunctionType.Exp)
                                if lk == 0:
                                    nc.gpsimd.tensor_mul(PT[:], PT[:],
                                                         mask_win[:])
                                elif lk == NWB - 1:
                                    nc.gpsimd.tensor_mul(PT[:], PT[:],
                                                         mask_causal[:])
                                nc.tensor.matmul(
                                    OT[:],
                                    vext[:, VP * g + VW * kb:
                                         VP * g + VW * (kb + 1)],
                                    PT[:], start=(lk == 0),
                                    stop=(lk == NWB - 1))
                            rcp = sm_pool.tile([1, REP * 128], DT,
                                               tag="rcp", name="rcp")
                            nc.vector.reciprocal(rcp[:], OT[64:65, :])
                            rcpb = sm_pool.tile([64, REP * 128], DT,
                                                tag="rcpb", name="rcpb")
                            nc.gpsimd.partition_broadcast(rcpb[:], rcp[:])
                            for r in range(REP):
                                h = REP * g + r
                                nc.vector.tensor_mul(
                                    aT[h // 2][64 * (h % 2):
                                               64 * (h % 2) + 64,
                                               128 * qt:128 * (qt + 1)],
                                    OT[0:64, 128 * r:128 * (r + 1)],
                                    rcpb[:, 128 * r:128 * (r + 1)])

                with tc.tile_pool(name="ps_q", bufs=1, space="PSUM") as ps_q:
                    for sweep in range(4):
                        q_quarter(sweep)
                        if sweep >= 1:
                            attention_pair(sweep - 1)
                attention_pair(3)

                # ================= output projection =================
                with (
                    tc.tile_pool(name="wo_s", bufs=10) as wo_pool,
                    tc.tile_pool(name="ostage", bufs=3) as ostage,
                    tc.tile_pool(name="ps_o", bufs=1, space="PSUM") as ps_o,
                ):
                    for oc in range(4):
                        ops = [ps_o.tile([128, 512], DT, tag=f"ops{tt}",
                                         name=f"ops{tt}")
                               for tt in range(NQT)]
                        for k in range(FD):
                            wot = wo_pool.tile([128, 512], F32R, tag="wo",
                                               name="wo")
                            eng = nc.scalar if k % 2 else nc.sync
                            eng.dma_start(
                                wot[:], wo_d[128 * k:128 * (k + 1),
                                             512 * oc:512 * (oc + 1)])
                            for tt in range(NQT):
                                nc.tensor.matmul(
                                    ops[tt][:],
                                    aT[k][:, 128 * tt:128 * (tt + 1)],
                                    wot[:], start=(k == 0),
                                    stop=(k == FD - 1))
                        for tt in range(NQT):
                            st = ostage.tile([128, 512], DT, tag="stage",
                                             name="stage")
                            nc.vector.tensor_copy(st[:], ops[tt][:])
                            nc.gpsimd.dma_start(
                                out_d[128 * tt:128 * (tt + 1),
                                      512 * oc:512 * (oc + 1)], st[:])

    nc.compile()
    return nc


def _rope_tables(t_idx, scale):
    """cos/sin tables in [d, t] layout, 2-head packed to 128 partitions.

    Rows 0-63 and 64-127 identical; sin rows 0-31 (and 64-95) carry the
    rotate-half minus sign."""
    inv_freq = 1.0 / (ROPE_BASE ** (np.arange(0, D, 2, dtype=np.float64) / D))
    ang = t_idx[None, :] * inv_freq[:, None]          # [32, n]
    cos1 = np.cos(ang)
    sin1 = np.sin(ang)
    cos64 = np.concatenate([cos1, cos1], 0) * scale   # [64, n]
    sin64 = np.concatenate([-sin1, sin1], 0) * scale  # [64, n] signed
    return (np.tile(cos64, (2, 1)).astype(np.float32),
            np.tile(sin64, (2, 1)).astype(np.float32))


def make_in_maps(x, Wq, Wk, Wv, Wo):
    x = np.asarray(x, np.float32)
    ins = []
    i = np.arange(128)
    masks = {
        "mask_win8": np.tile((i[:, None] > i[None, :]).astype(np.float32),
                             (1, REP)),
        "mask_causal8": np.tile((i[:, None] <= i[None, :]).astype(np.float32),
                                (1, REP)),
    }
    for c in range(NCORE):
        b, ch = divmod(c, 4)
        r0 = CH * ch
        kv0 = r0 - WIN
        xT = np.ascontiguousarray(x[b].T)             # [C, T]
        xkv = np.zeros((C, KVR), np.float32)
        pad = max(0, -kv0)
        xkv[:, pad:] = xT[:, kv0 + pad:r0 + CH]
        qc, qs = _rope_tables(np.arange(r0, r0 + CH, dtype=np.float64), SCALE)
        kc, ks = _rope_tables(np.arange(kv0, r0 + CH, dtype=np.float64), 1.0)
        kvvalid = np.zeros((128, NKB), np.float32)
        for lk in range(NKB):
            kvvalid[:, lk] = (kv0 + 128 * lk + i >= 0).astype(np.float32)
        ins.append({
            "xkv": xkv,
            "wq": np.ascontiguousarray(Wq, np.float32),
            "wk": np.ascontiguousarray(Wk, np.float32),
            "wv": np.ascontiguousarray(Wv, np.float32),
            "wo": np.ascontiguousarray(Wo, np.float32),
            "rope_q_cos": qc, "rope_q_sin": qs,
            "rope_k_cos": kc, "rope_k_sin": ks,
            "kvvalid": kvvalid,
            **masks,
        })
    return ins


_PROG_CACHE = {}


def get_program():
    if "nc" not in _PROG_CACHE:
        _PROG_CACHE["nc"] = build_program()
    return _PROG_CACHE["nc"]


def kernel(x, Wq, Wk, Wv, Wo):
    nc = get_program()
    ins = make_in_maps(x, Wq, Wk, Wv, Wo)
    res = run_bass_kernel_spmd(nc, ins, list(range(NCORE)))
    out = np.empty((B, T, C), np.float32)
    for c in range(NCORE):
        b, ch = divmod(c, 4)
        out[b, CH * ch:CH * (ch + 1), :] = res.results[c]["out"]
    return out
